# revision 10
# baseline (speedup 1.0000x reference)
"""Trainium2 Bass kernel: dense transformer attention block (QKV proj + RoPE +
GQA causal attention + output proj), tensor-parallel over 8 NeuronCores.

Sharding: heads split across cores (4 Q heads + 1 KV head per core). Each core
computes its QKV shard for all tokens, runs attention for its heads, the
head-sharded attention outputs are AllGathered in 256-token chunks (overlapped
with compute), and each core computes a 512-column slice of the output
projection.

v4: descriptor-pressure elimination. Hardware measurement shows the PE matmul
stream slows ~22% (216ns -> 263ns per 512-wide matmul) when the DMA engines
process many small (<=1KB line) descriptors concurrently; large (>=2KB)
descriptors at the same byte rate cost nothing. So v4:
  - Q and K never round-trip through HBM: the RoPE adds write straight into
    the SBUF attention tiles (kills ~14k 0.5-1KB descriptors).
  - The AllGather input/output buffers are d-major ([128 part, hh, t]), so
    the attention-output write is ONE dma of 2KB lines per query chunk and
    the o-proj gather reads are 2KB lines (was 512B).
  - The scalar queue carries no DMA while EXPs are in flight (EXPs were
    observed queued behind multi-us descriptor-gen stalls of strided DMAs).
Attention layout (kt-major, head-packed scores, EXP 2 heads per ACT op,
ones-matmul denominator broadcast, 0/1-mask multiplies, software-pipelined
score/PV emission) is unchanged from v3.
"""

from contextlib import ExitStack

import numpy as np
import ml_dtypes

import concourse.bass as bass
from concourse import bacc
import concourse.tile as tile
import concourse.mybir as mybir
from concourse.bass_utils import run_bass_kernel_spmd

F32 = mybir.dt.float32
F32R = mybir.dt.float32r
BF16 = mybir.dt.bfloat16
EXP = mybir.ActivationFunctionType.Exp

N_CORES = 8
N_HEADS = 32
N_KV_HEADS = 8
D = 128          # head dim
HID = 4096
B = 2
S = 2048
T = B * S        # 4096 tokens
ROPE_BASE = 10000.0

HL = N_HEADS // N_CORES          # 4 local Q heads per core
JC = HID // N_CORES              # 512 output columns per core

TC = 512                         # token chunk for the QKV projection phase
QC = 256                         # query chunk in attention
N_HT = HID // 128                # 32 hidden tiles
N_QC = S // QC                   # 8 query chunks per batch
OPW = 512                        # oproj token group width
N_OPG = T // OPW                 # 8 oproj groups
CKG = 4                          # 256-token chunks per AllGather

import os as _os
ABLATE_NO_CC = _os.environ.get("ABLATE_NO_CC", "0") == "1"


def _emit(tc_ctx, xt, wqkvt, wot, ropes, out_t, ag_ins, ag_outs):
    nc = tc_ctx.nc
    n_ch = T // TC               # 8 qkv chunks
    n_kt = S // 128              # 16 k-tiles per batch

    with ExitStack() as es:
        const_pool = es.enter_context(tc_ctx.tile_pool(name="const", bufs=1))
        # All-ones stationary: one matmul both sums over the key partition
        # axis and broadcasts the sums across all 128 partitions.
        ones_mat = const_pool.tile([128, 128], F32R)
        # Diagonal causal masks, replicated for a 2-head pack (slots
        # [2*d_off + hp]): maskrep[k, 2*d+hp, q] = 1.0 iff q - k - 128*d >= 0.
        maskrep = const_pool.tile([128, 4, QC], BF16)
        nc.vector.memset(maskrep, 1.0)
        # memset on a float32r tile fails the ISA check; copy from the
        # all-ones bf16 tile instead.
        nc.vector.tensor_copy(ones_mat, maskrep[:, 0, 0:128])
        for d_off in range(2):
            for hp in range(2):
                nc.gpsimd.affine_select(
                    out=maskrep[:, 2 * d_off + hp, :],
                    in_=maskrep[:, 2 * d_off + hp, :],
                    compare_op=mybir.AluOpType.is_ge,
                    fill=0.0,
                    base=-128 * d_off,
                    pattern=[[1, QC]],
                    channel_multiplier=-1,
                )
        # Warm the ACT exp table before attention needs it.
        act_warm = const_pool.tile([128, 1], F32)
        nc.scalar.activation(act_warm, ones_mat[:, 0:1], EXP)

        # Q/K/V live entirely in SBUF (written by phase A, read by attention).
        qpool = es.enter_context(tc_ctx.tile_pool(name="p2_q", bufs=4))
        kvpool = es.enter_context(tc_ctx.tile_pool(name="p2_kv", bufs=2))
        ps_s = es.enter_context(
            tc_ctx.tile_pool(name="p2_ps_s", bufs=2, space="PSUM"))
        ps_o = es.enter_context(
            tc_ctx.tile_pool(name="p2_ps_o", bufs=1, space="PSUM"))
        ps_b = es.enter_context(
            tc_ctx.tile_pool(name="p2_ps_b", bufs=1, space="PSUM"))
        ptpool = es.enter_context(tc_ctx.tile_pool(name="p2_pt", bufs=4))
        cspool = es.enter_context(tc_ctx.tile_pool(name="p2_cs", bufs=2))
        mpool = es.enter_context(tc_ctx.tile_pool(name="p2_misc", bufs=2))

        kvq = {}
        qtiles = {}

        def alloc_q(b, qc):
            q_t = qpool.tile([128, HL, QC], BF16, tag="q",
                             name=f"q_t{b}_{qc}")
            qtiles[(b, qc)] = q_t
            return q_t

        def alloc_kv(b):
            k_sb = kvpool.tile([128, S], BF16, tag="k", name=f"k_sb{b}")
            v_sb = kvpool.tile([128, n_kt, 128], BF16, tag="v",
                               name=f"v_sb{b}")
            kvq[b] = (k_sb, v_sb)

        def emit_attn(b, qc, filler=None):
            k_sb, v_sb = kvq[b]
            q_sb = qtiles.pop((b, qc))
            kt_max = 2 * qc + 2
            pso = [ps_o.tile([128, 2, QC], F32, tag=f"pso{hp}",
                             name=f"pso{hp}_{b}_{qc}")
                   for hp in range(2)]
            colsum = cspool.tile([128, 4, QC], F32R)
            pts = {}

            def emit_scores(kt, hp):
                # One N=512 matmul covers both heads of the pair.
                ps = ps_s.tile([128, 2, QC], F32)
                nc.tensor.matmul(
                    ps,
                    lhsT=k_sb[:, kt * 128:(kt + 1) * 128],
                    rhs=q_sb[:, hp * 2:hp * 2 + 2, :],
                    start=True,
                    stop=True,
                )
                pt = ptpool.tile([128, 2, QC], BF16)
                nc.scalar.activation(pt, ps, EXP)
                d_off = kt - 2 * qc
                if d_off >= 0:
                    nc.vector.tensor_mul(
                        pt, pt, maskrep[:, 2 * d_off:2 * d_off + 2, :]
                    )
                return pt

            def emit_pv(kt):
                for hp in range(2):
                    pt = pts.pop(kt)[hp] if hp == 1 else pts[kt][hp]
                    nc.tensor.matmul(
                        pso[hp],
                        lhsT=v_sb[:, kt, :],
                        rhs=pt,
                        start=(kt == 0),
                        stop=(kt == kt_max - 1),
                    )
                    if kt == 0:
                        nc.vector.tensor_copy(
                            colsum[:, hp * 2:hp * 2 + 2, :], pt
                        )
                    else:
                        nc.vector.tensor_add(
                            colsum[:, hp * 2:hp * 2 + 2, :],
                            colsum[:, hp * 2:hp * 2 + 2, :],
                            pt,
                        )

            for kt in range(kt_max):
                # Emit PV(kt-1) between the two score pairs of kt so the
                # in-order PE stream rides out the EXP latency with two
                # score-PSUM slots.
                cur = [emit_scores(kt, 0)]
                if kt >= 1:
                    emit_pv(kt - 1)
                cur.append(emit_scores(kt, 1))
                pts[kt] = cur
                if filler is not None and kt % 4 == 3:
                    # PE filler (o-proj blocks) while the serial EXP chain of
                    # the final pair catches up.
                    filler()
            emit_pv(kt_max - 1)

            # Partition-reduce + broadcast the denominators, then normalize
            # and ship to the AllGather input.
            sums_bc = ps_b.tile([128, 4, QC], F32)
            for hp in range(2):
                nc.tensor.matmul(
                    sums_bc[:, hp * 2:hp * 2 + 2, :],
                    lhsT=ones_mat,
                    rhs=colsum[:, hp * 2:hp * 2 + 2, :],
                    start=True,
                    stop=True,
                )
            recip = mpool.tile([128, 4, QC], F32, tag="recip")
            rscr = mpool.tile([128, 4, QC], F32, tag="rscr")
            nc.vector.reciprocal_approx_accurate(recip, sums_bc, rscr)
            ck = b * N_QC + qc
            attn_t = mpool.tile([128, 4, QC], BF16, tag="attn",
                                name=f"attn_{b}_{qc}")
            for hp in range(2):
                nc.vector.tensor_mul(
                    attn_t[:, hp * 2:hp * 2 + 2, :],
                    pso[hp], recip[:, hp * 2:hp * 2 + 2, :]
                )
            # d-major ag_in: one write, 2KB contiguous per partition.
            nc.sync.dma_start(out=ag_ins[ck // CKG][:, ck % CKG], in_=attn_t)
            if ck % CKG == CKG - 1 and not ABLATE_NO_CC:
                # One AllGather per CKG chunks: each collective carries
                # ~36us of fixed overhead on the CC stream, and CC-stream
                # activity throttles the PE clock ~20% while it runs.
                nc.gpsimd.collective_compute(
                    "AllGather",
                    mybir.AluOpType.bypass,
                    replica_groups=[list(range(N_CORES))],
                    ins=[ag_ins[ck // CKG][:]],
                    outs=[ag_outs[ck // CKG][:]],
                )

        def emit_attn_pair(pair):
            b, p = divmod(pair, 4)
            emit_attn(b, 2 * p)
            emit_attn(b, 2 * p + 1)

        # ------ Phase A: QKV projection + RoPE, attention interleaved -------
        with tc_ctx.tile_pool(name="p1_w", bufs=1) as wpool, \
             tc_ctx.tile_pool(name="p1_x", bufs=2) as xpool, \
             tc_ctx.tile_pool(name="p1_rope", bufs=2) as rpool, \
             tc_ctx.tile_pool(name="p1_ps", bufs=2, space="PSUM") as pspool, \
             tc_ctx.tile_pool(name="p1_sh", bufs=2) as shpool:
            wq_sb = wpool.tile([128, HL + 2, N_HT, 128], BF16)
            for ot in range(HL + 2):
                # Weights on the scalar-engine queue so the first X chunk
                # (sync queue) lands in parallel. Split ot=0 finely so the
                # very first matmuls start as soon as a slice arrives.
                if ot == 0:
                    for hq in range(4):
                        nc.scalar.dma_start(
                            out=wq_sb[:, 0, hq * 8:(hq + 1) * 8],
                            in_=wqkvt.ap()[:, 0, hq * 8:(hq + 1) * 8],
                        )
                else:
                    nc.scalar.dma_start(out=wq_sb[:, ot],
                                        in_=wqkvt.ap()[:, ot])
            for ch in range(n_ch):
                b, p = divmod(ch, S // TC)
                if p == 0:
                    alloc_kv(b)
                q_lo = alloc_q(b, 2 * p)
                q_hi = alloc_q(b, 2 * p + 1)
                x_sb = xpool.tile([128, N_HT, TC], BF16)
                if ch == 0:
                    splits = [0, 8, 16, 24, 32]
                else:
                    splits = [0, 16, 32]
                for si in range(len(splits) - 1):
                    lo, hi = splits[si], splits[si + 1]
                    nc.sync.dma_start(
                        out=x_sb[:, lo:hi, :],
                        in_=xt.ap()[:, ch, lo:hi, :],
                    )
                rope_sb = rpool.tile([128, 4, TC], BF16)
                nc.sync.dma_start(out=rope_sb, in_=ropes.ap()[:, ch])
                k_sb_b = None
                for ot in range(HL + 2):
                    if ot == HL + 1:
                        # V head, computed TRANSPOSED ([token, d]) by making
                        # the x slice the stationary operand: no HBM round
                        # trip and no DMA-transpose instructions.
                        psv = pspool.tile([128, 4, 128], F32, tag="ps")
                        for sub in range(4):
                            for h in range(N_HT):
                                nc.tensor.matmul(
                                    psv[:, sub, :],
                                    lhsT=x_sb[:, h,
                                              sub * 128:(sub + 1) * 128],
                                    rhs=wq_sb[:, ot, h, :],
                                    start=(sub == 0 and h == 0),
                                    stop=(sub == 3 and h == N_HT - 1),
                                )
                        v_sb = kvq[b][1]
                        nc.vector.tensor_copy(
                            v_sb[:, 4 * p:4 * p + 4, :], psv
                        )
                        continue
                    ps = pspool.tile([128, TC], F32, tag="ps")
                    for h in range(N_HT):
                        nc.tensor.matmul(
                            ps,
                            lhsT=wq_sb[:, ot, h, :],
                            rhs=x_sb[:, h, :],
                            start=(h == 0),
                            stop=(h == N_HT - 1),
                        )
                    # RoPE for Q (ot<HL, scaled tables) and K (ot==HL),
                    # written straight into the SBUF attention tiles.
                    ci = 0 if ot < HL else 2
                    # sh = rotate_half(ps) * sin  (sign folded into sin)
                    sh = shpool.tile([128, TC], F32, tag="sh")
                    nc.vector.tensor_mul(
                        sh[0:64, :], ps[64:128, :], rope_sb[0:64, ci + 1, :]
                    )
                    nc.vector.tensor_mul(
                        sh[64:128, :], ps[0:64, :],
                        rope_sb[64:128, ci + 1, :]
                    )
                    tmp = shpool.tile([128, TC], F32, tag="tmp")
                    nc.vector.tensor_mul(tmp, ps, rope_sb[:, ci, :])
                    if ot < HL:
                        nc.vector.tensor_add(
                            q_lo[:, ot, :], tmp[:, 0:QC], sh[:, 0:QC]
                        )
                        nc.vector.tensor_add(
                            q_hi[:, ot, :], tmp[:, QC:TC], sh[:, QC:TC]
                        )
                    else:
                        k_sb_b = kvq[b][0]
                        nc.vector.tensor_add(
                            k_sb_b[:, p * TC:(p + 1) * TC], tmp, sh
                        )
                # Chunk ch complete: run the attention pair that became
                # ready one chunk ago.
                if ch >= 1:
                    emit_attn_pair(ch - 1)

        # -------- Phase B: last attention pair + output projection ----------
        with tc_ctx.tile_pool(name="p4_w", bufs=1) as wopool, \
             tc_ctx.tile_pool(name="p4_a", bufs=2) as apool, \
             tc_ctx.tile_pool(name="p4_ag", bufs=4) as agpool, \
             tc_ctx.tile_pool(name="p4_ps", bufs=2, space="PSUM") as ps4pool:
            wo_sb = wopool.tile([128, N_HT, JC], BF16)
            nc.sync.dma_start(out=wo_sb, in_=wot.ap())

            def load_ag(g, eng):
                # One contiguous-tile DMA per 256-token chunk; each
                # (d, core) line is 2KB contiguous in the gathered buffer.
                tiles = []
                for sub in range(2):
                    ck = 2 * g + sub
                    ag_sb = agpool.tile([128, N_CORES, HL, QC], BF16, tag="ag",
                                        name=f"ag_sb{g}_{sub}")
                    eng.dma_start(
                        out=ag_sb,
                        in_=ag_outs[ck // CKG][:, :, ck % CKG].rearrange(
                            "c p hh t -> p c hh t"),
                    )
                    tiles.append(ag_sb)
                return tiles

            def emit_oproj_jt(g, ag_pair, jt):
                t0 = g * OPW
                ps4 = ps4pool.tile([128, 2, QC], F32, tag="ps4",
                                   name=f"ps4_{g}_{jt}")
                # Both 256-token halves accumulate into one PSUM bank: the
                # single start=True clears has_written for the whole bank;
                # each half's first matmul then writes via has_written==0.
                for h in range(N_HT):
                    for sub in range(2):
                        nc.tensor.matmul(
                            ps4[:, sub, :],
                            lhsT=wo_sb[:, h, jt * 128:(jt + 1) * 128],
                            rhs=ag_pair[sub][:, h // HL, h % HL, :],
                            start=(h == 0 and sub == 0),
                            stop=(h == N_HT - 1 and sub == 1),
                        )
                res4 = apool.tile([128, OPW], F32, tag="res4")
                nc.vector.tensor_copy(res4, ps4)
                nc.sync.dma_start(
                    out=out_t[jt * 128:(jt + 1) * 128, t0:t0 + OPW],
                    in_=res4,
                )

            # Prefetch the first two gathered groups (their AllGathers are
            # long done); their jt-blocks double as PE filler inside the
            # final attention pair, whose serial EXP chain otherwise starves
            # the PE. These loads stay on the sync queue: the last pair's
            # EXPs are still to come on the scalar queue.
            ag_tiles = {0: load_ag(0, nc.sync), 1: load_ag(1, nc.sync)}
            consumed = dict.fromkeys(range(N_OPG), 0)
            filler_q = [(g, jt) for g in range(2) for jt in range(JC // 128)]

            def filler():
                if filler_q:
                    g, jt = filler_q.pop(0)
                    emit_oproj_jt(g, ag_tiles[g], jt)
                    consumed[g] = jt + 1

            b7, p7 = divmod(n_ch - 1, 4)
            emit_attn(b7, 2 * p7, filler=filler)
            emit_attn(b7, 2 * p7 + 1, filler=filler)
            for g in range(N_OPG):
                if g + 2 < N_OPG:
                    # All EXPs are done by now; the scalar queue is free.
                    ag_tiles[g + 2] = load_ag(g + 2, nc.scalar)
                ag_sb = ag_tiles.pop(g)
                for jt in range(consumed[g], JC // 128):
                    emit_oproj_jt(g, ag_sb, jt)


def _build_program():
    nc = bacc.Bacc("TRN2", target_bir_lowering=False, debug=False,
                   num_devices=N_CORES)
    xt = nc.declare_dram_parameter("xt", [128, T // TC, N_HT, TC], BF16,
                                   isOutput=False)
    wqkvt = nc.declare_dram_parameter("wqkvt", [128, HL + 2, N_HT, 128], BF16,
                                      isOutput=False)
    wot = nc.declare_dram_parameter("wot", [128, N_HT, JC], BF16,
                                    isOutput=False)
    ropes = nc.declare_dram_parameter("ropes", [128, T // TC, 4, TC], BF16,
                                      isOutput=False)
    out_t = nc.declare_dram_parameter("out_t", [JC, T], F32, isOutput=True)

    # d-major collective buffers: ag_in[d, hh, t], gathered ag_out
    # [core, d, hh, t].
    ag_ins = [nc.dram_tensor(f"ag_in{k}", [128, CKG, HL, QC], BF16).ap()
              for k in range(T // QC // CKG)]
    ag_outs = [nc.dram_tensor(f"ag_out{k}", [N_CORES, 128, CKG, HL, QC],
                              BF16, addr_space="Shared").ap()
               for k in range(T // QC // CKG)]

    with tile.TileContext(nc) as tc_ctx:
        _emit(tc_ctx, xt, wqkvt, wot, ropes, out_t, ag_ins, ag_outs)
    nc.finalize()
    return nc


def _host_inputs(hidden_states, w_qkv, w_o):
    """Shard + transpose inputs for the 8 cores; returns in_maps."""
    X = np.asarray(hidden_states, dtype=np.float32).reshape(T, HID)
    # [p, ch, ht, tc] tiled layout so every DMA line is contiguous.
    xt = np.ascontiguousarray(
        X.reshape(T // TC, TC, N_HT, 128).transpose(3, 0, 2, 1)
    ).astype(ml_dtypes.bfloat16)

    # RoPE tables in [d, t] layout with rotate-half sign folded into sin and
    # the attention scale folded into the Q tables.
    inv_freq = 1.0 / (ROPE_BASE ** (np.arange(0, D, 2, dtype=np.float32) / D))
    pos = np.arange(S, dtype=np.float32)
    freqs = np.outer(pos, inv_freq)                      # (S, D/2)
    emb = np.concatenate([freqs, freqs], axis=-1)        # (S, D)
    cos = np.cos(emb).T.astype(np.float32)               # (D, S)
    sin = np.sin(emb).T.astype(np.float32)
    sgn = np.concatenate([-np.ones(D // 2), np.ones(D // 2)]).astype(np.float32)
    sins = sgn[:, None] * sin
    cos_t = np.tile(cos, (1, B))                         # (D, T)
    sins_t = np.tile(sins, (1, B))
    scale = np.float32(D ** -0.5)
    ropes = np.stack([cos_t * scale, sins_t * scale, cos_t, sins_t], axis=0)
    ropes = np.ascontiguousarray(
        ropes.reshape(4, 128, T // TC, TC).transpose(1, 2, 0, 3)
    ).astype(ml_dtypes.bfloat16)

    w_qkv = np.asarray(w_qkv, dtype=np.float32)
    w_o = np.asarray(w_o, dtype=np.float32)
    q_sz = N_HEADS * D
    kv_sz = N_KV_HEADS * D
    in_maps = []
    for c in range(N_CORES):
        qr = w_qkv[c * HL * D:(c + 1) * HL * D]
        kr = w_qkv[q_sz + c * D:q_sz + (c + 1) * D]
        vr = w_qkv[q_sz + kv_sz + c * D:q_sz + kv_sz + (c + 1) * D]
        w_shard = np.concatenate([qr, kr, vr], axis=0)           # (768, HID)
        wqkvt_c = np.ascontiguousarray(
            w_shard.reshape(HL + 2, 128, N_HT, 128).transpose(3, 0, 2, 1)
        ).astype(ml_dtypes.bfloat16)
        wot_c = np.ascontiguousarray(
            w_o[c * JC:(c + 1) * JC, :].reshape(JC, N_HT, 128).transpose(2, 1, 0)
        ).astype(ml_dtypes.bfloat16)
        in_maps.append({
            "xt": xt, "wqkvt": wqkvt_c, "wot": wot_c, "ropes": ropes,
        })
    return in_maps


def _run(hidden_states, w_qkv, w_o, trace=False, tmpdir=None):
    in_maps = _host_inputs(hidden_states, w_qkv, w_o)
    nc = _build_program()
    res = run_bass_kernel_spmd(nc, in_maps, list(range(N_CORES)),
                               trace=trace, tmpdir=tmpdir)
    out_T = np.concatenate(
        [np.asarray(res.results[c]["out_t"]) for c in range(N_CORES)], axis=0
    )                                                     # (HID j, T)
    out = np.ascontiguousarray(out_T.T).reshape(B, S, HID).astype(np.float32)
    return out, res


def kernel(hidden_states, w_qkv, w_o):
    out, _ = _run(hidden_states, w_qkv, w_o, trace=False)
    return out


# revision 13
# speedup vs baseline: 1.0492x; 1.0492x over previous
"""Trainium2 Bass kernel: dense transformer attention block (QKV proj + RoPE +
GQA causal attention + output proj), tensor-parallel over 8 NeuronCores.

Sharding: heads split across cores (4 Q heads + 1 KV head per core). Each core
computes its QKV shard for all tokens, runs attention for its heads, then a
PARTIAL output projection contracted over its own 512 attention rows for ALL
4096 output columns; the host sums the 8 fp32 partials. No on-device
collective at all.

v5: collective elimination. Hardware measurement shows a NEFF that contains
ANY collective runs its matmul stream at ~1.93 GHz for the whole execution,
while the identical stream without collectives sustains ~2.37 GHz (a latched
~22% clock penalty -- even one AllGather that completes in the first 100us
leaves the rest of the kernel throttled). Swapping the o-proj AllGather
(32MB/core gathered) for host-summed row-partials keeps FLOPs and weight
bytes identical, moves 64MB of fp32 partial writes per core (2KB lines,
~50GB/s, harmless), and restores the fast clock. The o-proj is interleaved
per attention pair, so phase B shrinks to the last pair + drain.

Also retained from v4: Q/K never round-trip through HBM (RoPE writes the
SBUF attention tiles directly), and all DMA stays off the scalar queue while
EXPs are in flight.
"""

from contextlib import ExitStack

import numpy as np
import ml_dtypes

import concourse.bass as bass
from concourse import bacc
import concourse.tile as tile
import concourse.mybir as mybir
from concourse.bass_utils import run_bass_kernel_spmd

F32 = mybir.dt.float32
F32R = mybir.dt.float32r
BF16 = mybir.dt.bfloat16
EXP = mybir.ActivationFunctionType.Exp

N_CORES = 8
N_HEADS = 32
N_KV_HEADS = 8
D = 128          # head dim
HID = 4096
B = 2
S = 2048
T = B * S        # 4096 tokens
ROPE_BASE = 10000.0

HL = N_HEADS // N_CORES          # 4 local Q heads per core

TC = 512                         # token chunk for the QKV projection phase
QC = 256                         # query chunk in attention
N_HT = HID // 128                # 32 hidden tiles
N_QC = S // QC                   # 8 query chunks per batch
N_JT = HID // 128                # 32 output-column tiles
N_CK = T // QC                   # 16 query chunks overall


def _emit(tc_ctx, xt, wqkvt, wot, ropes, out_t):
    nc = tc_ctx.nc
    n_ch = T // TC               # 8 qkv chunks
    n_kt = S // 128              # 16 k-tiles per batch

    with ExitStack() as es:
        const_pool = es.enter_context(tc_ctx.tile_pool(name="const", bufs=1))
        # All-ones stationary: one matmul both sums over the key partition
        # axis and broadcasts the sums across all 128 partitions.
        ones_mat = const_pool.tile([128, 128], F32R)
        # Diagonal causal masks, replicated for a 2-head pack (slots
        # [2*d_off + hp]): maskrep[k, 2*d+hp, q] = 1.0 iff q - k - 128*d >= 0.
        maskrep = const_pool.tile([128, 4, QC], BF16)
        nc.vector.memset(maskrep, 1.0)
        # memset on a float32r tile fails the ISA check; copy from the
        # all-ones bf16 tile instead.
        nc.vector.tensor_copy(ones_mat, maskrep[:, 0, 0:128])
        for d_off in range(2):
            for hp in range(2):
                nc.gpsimd.affine_select(
                    out=maskrep[:, 2 * d_off + hp, :],
                    in_=maskrep[:, 2 * d_off + hp, :],
                    compare_op=mybir.AluOpType.is_ge,
                    fill=0.0,
                    base=-128 * d_off,
                    pattern=[[1, QC]],
                    channel_multiplier=-1,
                )
        # Warm the ACT exp table before attention needs it.
        act_warm = const_pool.tile([128, 1], F32)
        nc.scalar.activation(act_warm, ones_mat[:, 0:1], EXP)

        # Q/K/V live entirely in SBUF (written by phase A, read by attention).
        qpool = es.enter_context(tc_ctx.tile_pool(name="p2_q", bufs=4))
        kvpool = es.enter_context(tc_ctx.tile_pool(name="p2_kv", bufs=2))
        ps_s = es.enter_context(
            tc_ctx.tile_pool(name="p2_ps_s", bufs=2, space="PSUM"))
        ps_o = es.enter_context(
            tc_ctx.tile_pool(name="p2_ps_o", bufs=1, space="PSUM"))
        ps_op = es.enter_context(
            tc_ctx.tile_pool(name="p3_ps", bufs=2, space="PSUM"))
        ptpool = es.enter_context(tc_ctx.tile_pool(name="p2_pt", bufs=4))
        cspool = es.enter_context(tc_ctx.tile_pool(name="p2_cs", bufs=2))
        mpool = es.enter_context(tc_ctx.tile_pool(name="p2_misc", bufs=1))
        atpool = es.enter_context(tc_ctx.tile_pool(name="p2_attn", bufs=2))
        respool = es.enter_context(tc_ctx.tile_pool(name="p3_res", bufs=2))
        wopool = es.enter_context(tc_ctx.tile_pool(name="p3_wo", bufs=1))

        # o-proj weights for this core's 512 attention rows, all 4096 cols.
        wo_sb = wopool.tile([128, HL, N_JT, 128], BF16)
        # Loaded on the scalar queue behind the qkv weights; first needed
        # at ~pair 0 (well after these land).
        nc.scalar.dma_start(out=wo_sb, in_=wot.ap())

        kvq = {}
        qtiles = {}
        attn_tiles = {}

        def alloc_q(b, qc):
            q_t = qpool.tile([128, HL, QC], BF16, tag="q",
                             name=f"q_t{b}_{qc}")
            qtiles[(b, qc)] = q_t
            return q_t

        def alloc_kv(b):
            k_sb = kvpool.tile([128, S], BF16, tag="k", name=f"k_sb{b}")
            v_sb = kvpool.tile([128, n_kt, 128], BF16, tag="v",
                               name=f"v_sb{b}")
            kvq[b] = (k_sb, v_sb)

        def emit_attn(b, qc):
            k_sb, v_sb = kvq[b]
            q_sb = qtiles.pop((b, qc))
            kt_max = 2 * qc + 2
            pso = [ps_o.tile([128, 2, QC], F32, tag=f"pso{hp}",
                             name=f"pso{hp}_{b}_{qc}")
                   for hp in range(2)]
            colsum = cspool.tile([128, 4, QC], F32R)
            pts = {}

            def emit_scores(kt, hp):
                # One N=512 matmul covers both heads of the pair.
                ps = ps_s.tile([128, 2, QC], F32)
                nc.tensor.matmul(
                    ps,
                    lhsT=k_sb[:, kt * 128:(kt + 1) * 128],
                    rhs=q_sb[:, hp * 2:hp * 2 + 2, :],
                    start=True,
                    stop=True,
                )
                pt = ptpool.tile([128, 2, QC], BF16)
                nc.scalar.activation(pt, ps, EXP)
                d_off = kt - 2 * qc
                if d_off >= 0:
                    nc.vector.tensor_mul(
                        pt, pt, maskrep[:, 2 * d_off:2 * d_off + 2, :]
                    )
                return pt

            def emit_pv(kt):
                for hp in range(2):
                    pt = pts.pop(kt)[hp] if hp == 1 else pts[kt][hp]
                    nc.tensor.matmul(
                        pso[hp],
                        lhsT=v_sb[:, kt, :],
                        rhs=pt,
                        start=(kt == 0),
                        stop=(kt == kt_max - 1),
                    )
                    if kt == 0:
                        nc.vector.tensor_copy(
                            colsum[:, hp * 2:hp * 2 + 2, :], pt
                        )
                    else:
                        nc.vector.tensor_add(
                            colsum[:, hp * 2:hp * 2 + 2, :],
                            colsum[:, hp * 2:hp * 2 + 2, :],
                            pt,
                        )

            for kt in range(kt_max):
                # Emit PV(kt-1) between the two score pairs of kt so the
                # in-order PE stream rides out the EXP latency with two
                # score-PSUM slots.
                cur = [emit_scores(kt, 0)]
                if kt >= 1:
                    emit_pv(kt - 1)
                cur.append(emit_scores(kt, 1))
                pts[kt] = cur
            emit_pv(kt_max - 1)

            # Partition-reduce + broadcast the denominators (into the score
            # PSUM pool -- scores are drained by now), then normalize.
            sums_bc = [ps_s.tile([128, 2, QC], F32, tag="ps",
                                 name=f"sums{hp}_{b}_{qc}")
                       for hp in range(2)]
            for hp in range(2):
                nc.tensor.matmul(
                    sums_bc[hp],
                    lhsT=ones_mat,
                    rhs=colsum[:, hp * 2:hp * 2 + 2, :],
                    start=True,
                    stop=True,
                )
            recip = mpool.tile([128, 4, QC], F32, tag="recip")
            rscr = mpool.tile([128, 4, QC], F32, tag="rscr")
            for hp in range(2):
                nc.vector.reciprocal_approx_accurate(
                    recip[:, hp * 2:hp * 2 + 2, :], sums_bc[hp],
                    rscr[:, hp * 2:hp * 2 + 2, :])
            attn_t = atpool.tile([128, HL, QC], BF16, tag="attn",
                                 name=f"attn_{b}_{qc}")
            for hp in range(2):
                nc.vector.tensor_mul(
                    attn_t[:, hp * 2:hp * 2 + 2, :],
                    pso[hp], recip[:, hp * 2:hp * 2 + 2, :]
                )
            attn_tiles[(b, qc)] = attn_t

        def emit_oproj_pair(pair):
            # Partial o-proj for this pair's 512 tokens, contracted over the
            # core's 512 attention rows (4 head-tiles x 2 query chunks).
            b, p = divmod(pair, 4)
            at = [attn_tiles.pop((b, 2 * p)), attn_tiles.pop((b, 2 * p + 1))]
            for jt in range(N_JT):
                po = ps_op.tile([128, 2, QC], F32, tag="po",
                                name=f"po_{pair}_{jt}")
                for hh in range(HL):
                    for sub in range(2):
                        nc.tensor.matmul(
                            po[:, sub, :],
                            lhsT=wo_sb[:, hh, jt, :],
                            rhs=at[sub][:, hh, :],
                            start=(hh == 0 and sub == 0),
                            stop=(hh == HL - 1 and sub == 1),
                        )
                res = respool.tile([128, 2, QC], F32, tag="res",
                                   name=f"res_{pair}_{jt}")
                nc.vector.tensor_copy(res, po)
                # 2KB lines: [128 j, 2 chunks, 256 t] fp32 per write.
                nc.sync.dma_start(
                    out=out_t.ap()[jt, :, 2 * pair:2 * pair + 2, :],
                    in_=res,
                )

        def emit_pair(pair):
            b, p = divmod(pair, 4)
            emit_attn(b, 2 * p)
            emit_attn(b, 2 * p + 1)
            emit_oproj_pair(pair)

        # ------ Phase A: QKV projection + RoPE, attention + o-proj
        # interleaved one pair behind -------
        with tc_ctx.tile_pool(name="p1_w", bufs=1) as wpool, \
             tc_ctx.tile_pool(name="p1_x", bufs=3) as xpool, \
             tc_ctx.tile_pool(name="p1_rope", bufs=2) as rpool, \
             tc_ctx.tile_pool(name="p1_ps", bufs=2, space="PSUM") as pspool, \
             tc_ctx.tile_pool(name="p1_sh", bufs=1) as shpool:
            wq_sb = wpool.tile([128, HL + 2, N_HT, 128], BF16)
            for ot in range(HL + 2):
                # Weights on the scalar-engine queue so the first X chunk
                # (sync queue) lands in parallel. Split ot=0 finely so the
                # very first matmuls start as soon as a slice arrives.
                if ot == 0:
                    for hq in range(4):
                        nc.scalar.dma_start(
                            out=wq_sb[:, 0, hq * 8:(hq + 1) * 8],
                            in_=wqkvt.ap()[:, 0, hq * 8:(hq + 1) * 8],
                        )
                else:
                    nc.scalar.dma_start(out=wq_sb[:, ot],
                                        in_=wqkvt.ap()[:, ot])
            for ch in range(n_ch):
                b, p = divmod(ch, S // TC)
                if p == 0:
                    alloc_kv(b)
                q_lo = alloc_q(b, 2 * p)
                q_hi = alloc_q(b, 2 * p + 1)
                # x in two 16-ht halves (16KB/partition each, triple
                # buffered) to fit SBUF alongside the o-proj weights.
                x_half = []
                for half in range(2):
                    xh = xpool.tile([128, N_HT // 2, TC], BF16, tag="x",
                                    name=f"x_{ch}_{half}")
                    lo = half * (N_HT // 2)
                    if ch == 0 and half == 0:
                        for piece in range(2):
                            nc.sync.dma_start(
                                out=xh[:, piece * 8:(piece + 1) * 8, :],
                                in_=xt.ap()[:, ch,
                                            lo + piece * 8:lo + (piece + 1) * 8,
                                            :],
                            )
                    else:
                        nc.sync.dma_start(out=xh,
                                          in_=xt.ap()[:, ch, lo:lo + 16, :])
                    x_half.append(xh)

                def x_sl(h):
                    return x_half[h // 16][:, h % 16, :]

                rope_sb = rpool.tile([128, 4, TC], BF16)
                nc.sync.dma_start(out=rope_sb, in_=ropes.ap()[:, ch])
                for ot in range(HL + 2):
                    if ot == HL + 1:
                        # V head, computed TRANSPOSED ([token, d]) by making
                        # the x slice the stationary operand: no HBM round
                        # trip and no DMA-transpose instructions.
                        psv = pspool.tile([128, 4, 128], F32, tag="ps")
                        for sub in range(4):
                            for h in range(N_HT):
                                nc.tensor.matmul(
                                    psv[:, sub, :],
                                    lhsT=x_sl(h)[:, sub * 128:(sub + 1) * 128],
                                    rhs=wq_sb[:, ot, h, :],
                                    start=(sub == 0 and h == 0),
                                    stop=(sub == 3 and h == N_HT - 1),
                                )
                        v_sb = kvq[b][1]
                        nc.vector.tensor_copy(
                            v_sb[:, 4 * p:4 * p + 4, :], psv
                        )
                        continue
                    ps = pspool.tile([128, TC], F32, tag="ps")
                    for h in range(N_HT):
                        nc.tensor.matmul(
                            ps,
                            lhsT=wq_sb[:, ot, h, :],
                            rhs=x_sl(h),
                            start=(h == 0),
                            stop=(h == N_HT - 1),
                        )
                    # RoPE for Q (ot<HL, scaled tables) and K (ot==HL),
                    # written straight into the SBUF attention tiles.
                    ci = 0 if ot < HL else 2
                    # sh = rotate_half(ps) * sin  (sign folded into sin)
                    sh = shpool.tile([128, TC], F32, tag="sh")
                    nc.vector.tensor_mul(
                        sh[0:64, :], ps[64:128, :], rope_sb[0:64, ci + 1, :]
                    )
                    nc.vector.tensor_mul(
                        sh[64:128, :], ps[0:64, :],
                        rope_sb[64:128, ci + 1, :]
                    )
                    tmp = shpool.tile([128, TC], F32, tag="tmp")
                    nc.vector.tensor_mul(tmp, ps, rope_sb[:, ci, :])
                    if ot < HL:
                        nc.vector.tensor_add(
                            q_lo[:, ot, :], tmp[:, 0:QC], sh[:, 0:QC]
                        )
                        nc.vector.tensor_add(
                            q_hi[:, ot, :], tmp[:, QC:TC], sh[:, QC:TC]
                        )
                    else:
                        k_sb_b = kvq[b][0]
                        nc.vector.tensor_add(
                            k_sb_b[:, p * TC:(p + 1) * TC], tmp, sh
                        )
                # Chunk ch complete: run attention + o-proj for the pair
                # that became ready one chunk ago.
                if ch >= 1:
                    emit_pair(ch - 1)

        # -------- Phase B: just the last pair ----------
        emit_pair(n_ch - 1)


def _build_program():
    nc = bacc.Bacc("TRN2", target_bir_lowering=False, debug=False,
                   num_devices=N_CORES)
    xt = nc.declare_dram_parameter("xt", [128, T // TC, N_HT, TC], BF16,
                                   isOutput=False)
    wqkvt = nc.declare_dram_parameter("wqkvt", [128, HL + 2, N_HT, 128], BF16,
                                      isOutput=False)
    wot = nc.declare_dram_parameter("wot", [128, HL, N_JT, 128], BF16,
                                    isOutput=False)
    ropes = nc.declare_dram_parameter("ropes", [128, T // TC, 4, TC], BF16,
                                      isOutput=False)
    # fp32 partial o-proj: [jt, j-in-tile, query-chunk, t]; host sums cores.
    out_t = nc.declare_dram_parameter("out_t", [N_JT, 128, N_CK, QC], F32,
                                      isOutput=True)

    with tile.TileContext(nc) as tc_ctx:
        _emit(tc_ctx, xt, wqkvt, wot, ropes, out_t)
    nc.finalize()
    return nc


def _host_inputs(hidden_states, w_qkv, w_o):
    """Shard + transpose inputs for the 8 cores; returns in_maps."""
    X = np.asarray(hidden_states, dtype=np.float32).reshape(T, HID)
    # [p, ch, ht, tc] tiled layout so every DMA line is contiguous.
    xt = np.ascontiguousarray(
        X.reshape(T // TC, TC, N_HT, 128).transpose(3, 0, 2, 1)
    ).astype(ml_dtypes.bfloat16)

    # RoPE tables in [d, t] layout with rotate-half sign folded into sin and
    # the attention scale folded into the Q tables.
    inv_freq = 1.0 / (ROPE_BASE ** (np.arange(0, D, 2, dtype=np.float32) / D))
    pos = np.arange(S, dtype=np.float32)
    freqs = np.outer(pos, inv_freq)                      # (S, D/2)
    emb = np.concatenate([freqs, freqs], axis=-1)        # (S, D)
    cos = np.cos(emb).T.astype(np.float32)               # (D, S)
    sin = np.sin(emb).T.astype(np.float32)
    sgn = np.concatenate([-np.ones(D // 2), np.ones(D // 2)]).astype(np.float32)
    sins = sgn[:, None] * sin
    cos_t = np.tile(cos, (1, B))                         # (D, T)
    sins_t = np.tile(sins, (1, B))
    scale = np.float32(D ** -0.5)
    ropes = np.stack([cos_t * scale, sins_t * scale, cos_t, sins_t], axis=0)
    ropes = np.ascontiguousarray(
        ropes.reshape(4, 128, T // TC, TC).transpose(1, 2, 0, 3)
    ).astype(ml_dtypes.bfloat16)

    w_qkv = np.asarray(w_qkv, dtype=np.float32)
    w_o = np.asarray(w_o, dtype=np.float32)
    q_sz = N_HEADS * D
    kv_sz = N_KV_HEADS * D
    in_maps = []
    for c in range(N_CORES):
        qr = w_qkv[c * HL * D:(c + 1) * HL * D]
        kr = w_qkv[q_sz + c * D:q_sz + (c + 1) * D]
        vr = w_qkv[q_sz + kv_sz + c * D:q_sz + kv_sz + (c + 1) * D]
        w_shard = np.concatenate([qr, kr, vr], axis=0)           # (768, HID)
        wqkvt_c = np.ascontiguousarray(
            w_shard.reshape(HL + 2, 128, N_HT, 128).transpose(3, 0, 2, 1)
        ).astype(ml_dtypes.bfloat16)
        # o-proj slice: this core's 512 attention rows, all 4096 columns,
        # laid out [d-part, hh, jt, j].
        wo_rows = w_o[:, c * HL * D:(c + 1) * HL * D]            # (4096, 512)
        wot_c = np.ascontiguousarray(
            wo_rows.T.reshape(HL, 128, N_JT, 128).transpose(1, 0, 2, 3)
        ).astype(ml_dtypes.bfloat16)
        in_maps.append({
            "xt": xt, "wqkvt": wqkvt_c, "wot": wot_c, "ropes": ropes,
        })
    return in_maps


def _run(hidden_states, w_qkv, w_o, trace=False, tmpdir=None):
    in_maps = _host_inputs(hidden_states, w_qkv, w_o)
    nc = _build_program()
    res = run_bass_kernel_spmd(nc, in_maps, list(range(N_CORES)),
                               trace=trace, tmpdir=tmpdir)
    acc = np.zeros((N_JT, 128, N_CK, QC), dtype=np.float32)
    for c in range(N_CORES):
        acc += np.asarray(res.results[c]["out_t"])
    # [jt, j, ck, t] -> [j(4096), t(4096)] -> [B, S, HID]
    out_jt = acc.reshape(N_JT * 128, N_CK * QC)
    out = np.ascontiguousarray(out_jt.T).reshape(B, S, HID).astype(np.float32)
    return out, res


def kernel(hidden_states, w_qkv, w_o):
    out, _ = _run(hidden_states, w_qkv, w_o, trace=False)
    return out


# revision 14
# speedup vs baseline: 1.0620x; 1.0122x over previous
"""Trainium2 Bass kernel: dense transformer attention block (QKV proj + RoPE +
GQA causal attention + output proj), tensor-parallel over 8 NeuronCores.

Sharding: heads split across cores (4 Q heads + 1 KV head per core). Each core
computes its QKV shard for all tokens, runs attention for its heads, then a
PARTIAL output projection contracted over its own 512 attention rows for ALL
4096 output columns; the host sums the 8 fp32 partials. No on-device
collective at all.

v5: collective elimination. Hardware measurement shows a NEFF that contains
ANY collective runs its matmul stream at ~1.93 GHz for the whole execution,
while the identical stream without collectives sustains ~2.37 GHz (a latched
~22% clock penalty -- even one AllGather that completes in the first 100us
leaves the rest of the kernel throttled). Swapping the o-proj AllGather
(32MB/core gathered) for host-summed row-partials keeps FLOPs and weight
bytes identical, moves 64MB of fp32 partial writes per core (2KB lines,
~50GB/s, harmless), and restores the fast clock. The o-proj is interleaved
per attention pair, so phase B shrinks to the last pair + drain.

Also retained from v4: Q/K never round-trip through HBM (RoPE writes the
SBUF attention tiles directly), and all DMA stays off the scalar queue while
EXPs are in flight.
"""

from contextlib import ExitStack

import numpy as np
import ml_dtypes

import concourse.bass as bass
from concourse import bacc
import concourse.tile as tile
import concourse.mybir as mybir
from concourse.bass_utils import run_bass_kernel_spmd

F32 = mybir.dt.float32
F32R = mybir.dt.float32r
BF16 = mybir.dt.bfloat16
EXP = mybir.ActivationFunctionType.Exp

N_CORES = 8
N_HEADS = 32
N_KV_HEADS = 8
D = 128          # head dim
HID = 4096
B = 2
S = 2048
T = B * S        # 4096 tokens
ROPE_BASE = 10000.0

HL = N_HEADS // N_CORES          # 4 local Q heads per core

TC = 512                         # token chunk for the QKV projection phase
QC = 256                         # query chunk in attention
N_HT = HID // 128                # 32 hidden tiles
N_QC = S // QC                   # 8 query chunks per batch
N_JT = HID // 128                # 32 output-column tiles
N_CK = T // QC                   # 16 query chunks overall


def _emit(tc_ctx, xt, wqkvt, wot, ropes, out_t):
    nc = tc_ctx.nc
    n_ch = T // TC               # 8 qkv chunks
    n_kt = S // 128              # 16 k-tiles per batch

    with ExitStack() as es:
        const_pool = es.enter_context(tc_ctx.tile_pool(name="const", bufs=1))
        # All-ones stationary: one matmul both sums over the key partition
        # axis and broadcasts the sums across all 128 partitions.
        ones_mat = const_pool.tile([128, 128], F32R)
        # Diagonal causal masks, replicated for a 2-head pack (slots
        # [2*d_off + hp]): maskrep[k, 2*d+hp, q] = 1.0 iff q - k - 128*d >= 0.
        maskrep = const_pool.tile([128, 4, QC], BF16)
        nc.vector.memset(maskrep, 1.0)
        # memset on a float32r tile fails the ISA check; copy from the
        # all-ones bf16 tile instead.
        nc.vector.tensor_copy(ones_mat, maskrep[:, 0, 0:128])
        for d_off in range(2):
            for hp in range(2):
                nc.gpsimd.affine_select(
                    out=maskrep[:, 2 * d_off + hp, :],
                    in_=maskrep[:, 2 * d_off + hp, :],
                    compare_op=mybir.AluOpType.is_ge,
                    fill=0.0,
                    base=-128 * d_off,
                    pattern=[[1, QC]],
                    channel_multiplier=-1,
                )
        # Warm the ACT exp table before attention needs it.
        act_warm = const_pool.tile([128, 1], F32)
        nc.scalar.activation(act_warm, ones_mat[:, 0:1], EXP)

        # Q/K/V live entirely in SBUF (written by phase A, read by attention).
        qpool = es.enter_context(tc_ctx.tile_pool(name="p2_q", bufs=4))
        kvpool = es.enter_context(tc_ctx.tile_pool(name="p2_kv", bufs=2))
        ps_s = es.enter_context(
            tc_ctx.tile_pool(name="p2_ps_s", bufs=2, space="PSUM"))
        ps_o = es.enter_context(
            tc_ctx.tile_pool(name="p2_ps_o", bufs=1, space="PSUM"))
        ps_op = es.enter_context(
            tc_ctx.tile_pool(name="p3_ps", bufs=2, space="PSUM"))
        ptpool = es.enter_context(tc_ctx.tile_pool(name="p2_pt", bufs=4))
        cspool = es.enter_context(tc_ctx.tile_pool(name="p2_cs", bufs=2))
        mpool = es.enter_context(tc_ctx.tile_pool(name="p2_misc", bufs=1))
        atpool = es.enter_context(tc_ctx.tile_pool(name="p2_attn", bufs=2))
        respool = es.enter_context(tc_ctx.tile_pool(name="p3_res", bufs=2))
        wopool = es.enter_context(tc_ctx.tile_pool(name="p3_wo", bufs=1))

        # o-proj weights for this core's 512 attention rows, all 4096 cols.
        wo_sb = wopool.tile([128, HL, N_JT, 128], BF16)
        # Loaded on the scalar queue behind the qkv weights; first needed
        # at ~pair 0 (well after these land).
        nc.scalar.dma_start(out=wo_sb, in_=wot.ap())

        kvq = {}
        qtiles = {}
        attn_tiles = {}

        def alloc_q(b, qc):
            q_t = qpool.tile([128, HL, QC], BF16, tag="q",
                             name=f"q_t{b}_{qc}")
            qtiles[(b, qc)] = q_t
            return q_t

        def alloc_kv(b):
            k_sb = kvpool.tile([128, S], BF16, tag="k", name=f"k_sb{b}")
            v_sb = kvpool.tile([128, n_kt, 128], BF16, tag="v",
                               name=f"v_sb{b}")
            kvq[b] = (k_sb, v_sb)

        def emit_attn(b, qc, at_pair, sub):
            k_sb, v_sb = kvq[b]
            q_sb = qtiles.pop((b, qc))
            kt_max = 2 * qc + 2
            pso = [ps_o.tile([128, 2, QC], F32, tag=f"pso{hp}",
                             name=f"pso{hp}_{b}_{qc}")
                   for hp in range(2)]
            colsum = cspool.tile([128, 4, QC], F32R)
            pts = {}

            def emit_scores(kt, hp):
                # One N=512 matmul covers both heads of the pair.
                ps = ps_s.tile([128, 2, QC], F32)
                nc.tensor.matmul(
                    ps,
                    lhsT=k_sb[:, kt * 128:(kt + 1) * 128],
                    rhs=q_sb[:, hp * 2:hp * 2 + 2, :],
                    start=True,
                    stop=True,
                )
                pt = ptpool.tile([128, 2, QC], BF16)
                nc.scalar.activation(pt, ps, EXP)
                d_off = kt - 2 * qc
                if d_off >= 0:
                    nc.vector.tensor_mul(
                        pt, pt, maskrep[:, 2 * d_off:2 * d_off + 2, :]
                    )
                return pt

            def emit_pv(kt):
                for hp in range(2):
                    pt = pts.pop(kt)[hp] if hp == 1 else pts[kt][hp]
                    nc.tensor.matmul(
                        pso[hp],
                        lhsT=v_sb[:, kt, :],
                        rhs=pt,
                        start=(kt == 0),
                        stop=(kt == kt_max - 1),
                    )
                    if kt == 0:
                        nc.vector.tensor_copy(
                            colsum[:, hp * 2:hp * 2 + 2, :], pt
                        )
                    else:
                        nc.vector.tensor_add(
                            colsum[:, hp * 2:hp * 2 + 2, :],
                            colsum[:, hp * 2:hp * 2 + 2, :],
                            pt,
                        )

            for kt in range(kt_max):
                # Emit PV(kt-1) between the two score pairs of kt so the
                # in-order PE stream rides out the EXP latency with two
                # score-PSUM slots.
                cur = [emit_scores(kt, 0)]
                if kt >= 1:
                    emit_pv(kt - 1)
                cur.append(emit_scores(kt, 1))
                pts[kt] = cur
            emit_pv(kt_max - 1)

            # Partition-reduce + broadcast the denominators (into the score
            # PSUM pool -- scores are drained by now), then normalize.
            sums_bc = [ps_s.tile([128, 2, QC], F32, tag="ps",
                                 name=f"sums{hp}_{b}_{qc}")
                       for hp in range(2)]
            for hp in range(2):
                nc.tensor.matmul(
                    sums_bc[hp],
                    lhsT=ones_mat,
                    rhs=colsum[:, hp * 2:hp * 2 + 2, :],
                    start=True,
                    stop=True,
                )
            recip = mpool.tile([128, 4, QC], F32, tag="recip")
            rscr = mpool.tile([128, 4, QC], F32, tag="rscr")
            for hp in range(2):
                nc.vector.reciprocal_approx_accurate(
                    recip[:, hp * 2:hp * 2 + 2, :], sums_bc[hp],
                    rscr[:, hp * 2:hp * 2 + 2, :])
            for hp in range(2):
                nc.vector.tensor_mul(
                    at_pair[:, hp * 2:hp * 2 + 2, sub, :],
                    pso[hp], recip[:, hp * 2:hp * 2 + 2, :]
                )

        def emit_oproj_pair(pair):
            # Partial o-proj for this pair's 512 tokens, contracted over the
            # core's 512 attention rows (4 head-tiles x 2 query chunks).
            at_pair = attn_tiles.pop(pair)
            for jt in range(N_JT):
                po = ps_op.tile([128, 2, QC], F32, tag="po",
                                name=f"po_{pair}_{jt}")
                for hh in range(HL):
                    nc.tensor.matmul(
                        po,
                        lhsT=wo_sb[:, hh, jt, :],
                        rhs=at_pair[:, hh],
                        start=(hh == 0),
                        stop=(hh == HL - 1),
                    )
                res = respool.tile([128, 2, QC], F32, tag="res",
                                   name=f"res_{pair}_{jt}")
                nc.vector.tensor_copy(res, po)
                # 2KB lines: [128 j, 2 chunks, 256 t] fp32 per write.
                nc.sync.dma_start(
                    out=out_t.ap()[jt, :, 2 * pair:2 * pair + 2, :],
                    in_=res,
                )

        def emit_attn_pair(pair):
            b, p = divmod(pair, 4)
            at_pair = atpool.tile([128, HL, 2, QC], BF16, tag="attn",
                                  name=f"attn_{pair}")
            attn_tiles[pair] = at_pair
            emit_attn(b, 2 * p, at_pair, 0)
            emit_attn(b, 2 * p + 1, at_pair, 1)

        # ------ Phase A: QKV projection + RoPE, attention + o-proj
        # interleaved one pair behind -------
        with tc_ctx.tile_pool(name="p1_w", bufs=1) as wpool, \
             tc_ctx.tile_pool(name="p1_x", bufs=3) as xpool, \
             tc_ctx.tile_pool(name="p1_rope", bufs=2) as rpool, \
             tc_ctx.tile_pool(name="p1_ps", bufs=2, space="PSUM") as pspool, \
             tc_ctx.tile_pool(name="p1_sh", bufs=1) as shpool:
            wq_sb = wpool.tile([128, HL + 2, N_HT, 128], BF16)
            for ot in range(HL + 2):
                # Weights on the scalar-engine queue so the first X chunk
                # (sync queue) lands in parallel. Split ot=0 finely so the
                # very first matmuls start as soon as a slice arrives.
                if ot == 0:
                    for hq in range(4):
                        nc.scalar.dma_start(
                            out=wq_sb[:, 0, hq * 8:(hq + 1) * 8],
                            in_=wqkvt.ap()[:, 0, hq * 8:(hq + 1) * 8],
                        )
                else:
                    nc.scalar.dma_start(out=wq_sb[:, ot],
                                        in_=wqkvt.ap()[:, ot])
            for ch in range(n_ch):
                b, p = divmod(ch, S // TC)
                if p == 0:
                    alloc_kv(b)
                q_lo = alloc_q(b, 2 * p)
                q_hi = alloc_q(b, 2 * p + 1)
                # x in two 16-ht halves (16KB/partition each, triple
                # buffered) to fit SBUF alongside the o-proj weights.
                x_half = []
                for half in range(2):
                    xh = xpool.tile([128, N_HT // 2, TC], BF16, tag="x",
                                    name=f"x_{ch}_{half}")
                    lo = half * (N_HT // 2)
                    if ch == 0 and half == 0:
                        for piece in range(2):
                            nc.sync.dma_start(
                                out=xh[:, piece * 8:(piece + 1) * 8, :],
                                in_=xt.ap()[:, ch,
                                            lo + piece * 8:lo + (piece + 1) * 8,
                                            :],
                            )
                    else:
                        nc.sync.dma_start(out=xh,
                                          in_=xt.ap()[:, ch, lo:lo + 16, :])
                    x_half.append(xh)

                def x_sl(h):
                    return x_half[h // 16][:, h % 16, :]

                rope_sb = rpool.tile([128, 4, TC], BF16)
                nc.sync.dma_start(out=rope_sb, in_=ropes.ap()[:, ch])
                for ot in range(HL + 2):
                    if ot == 1 and ch >= 2:
                        # o-proj of the pair finished during the previous
                        # chunk: emitted here, behind the first QKV chain, so
                        # the PE never waits on that pair's normalize chain.
                        emit_oproj_pair(ch - 2)
                    if ot == HL + 1:
                        # V head, computed TRANSPOSED ([token, d]) by making
                        # the x slice the stationary operand: no HBM round
                        # trip and no DMA-transpose instructions.
                        psv = pspool.tile([128, 4, 128], F32, tag="ps")
                        for sub in range(4):
                            for h in range(N_HT):
                                nc.tensor.matmul(
                                    psv[:, sub, :],
                                    lhsT=x_sl(h)[:, sub * 128:(sub + 1) * 128],
                                    rhs=wq_sb[:, ot, h, :],
                                    start=(sub == 0 and h == 0),
                                    stop=(sub == 3 and h == N_HT - 1),
                                )
                        v_sb = kvq[b][1]
                        nc.vector.tensor_copy(
                            v_sb[:, 4 * p:4 * p + 4, :], psv
                        )
                        continue
                    ps = pspool.tile([128, TC], F32, tag="ps")
                    for h in range(N_HT):
                        nc.tensor.matmul(
                            ps,
                            lhsT=wq_sb[:, ot, h, :],
                            rhs=x_sl(h),
                            start=(h == 0),
                            stop=(h == N_HT - 1),
                        )
                    # RoPE for Q (ot<HL, scaled tables) and K (ot==HL),
                    # written straight into the SBUF attention tiles.
                    ci = 0 if ot < HL else 2
                    # sh = rotate_half(ps) * sin  (sign folded into sin)
                    sh = shpool.tile([128, TC], F32, tag="sh")
                    nc.vector.tensor_mul(
                        sh[0:64, :], ps[64:128, :], rope_sb[0:64, ci + 1, :]
                    )
                    nc.vector.tensor_mul(
                        sh[64:128, :], ps[0:64, :],
                        rope_sb[64:128, ci + 1, :]
                    )
                    tmp = shpool.tile([128, TC], F32, tag="tmp")
                    nc.vector.tensor_mul(tmp, ps, rope_sb[:, ci, :])
                    if ot < HL:
                        nc.vector.tensor_add(
                            q_lo[:, ot, :], tmp[:, 0:QC], sh[:, 0:QC]
                        )
                        nc.vector.tensor_add(
                            q_hi[:, ot, :], tmp[:, QC:TC], sh[:, QC:TC]
                        )
                    else:
                        k_sb_b = kvq[b][0]
                        nc.vector.tensor_add(
                            k_sb_b[:, p * TC:(p + 1) * TC], tmp, sh
                        )
                # Chunk ch complete: run attention for the pair that
                # became ready one chunk ago.
                if ch >= 1:
                    emit_attn_pair(ch - 1)

        # -------- Phase B: pending o-proj + the last pair ----------
        emit_oproj_pair(n_ch - 2)
        emit_attn_pair(n_ch - 1)
        emit_oproj_pair(n_ch - 1)


def _build_program():
    nc = bacc.Bacc("TRN2", target_bir_lowering=False, debug=False,
                   num_devices=N_CORES)
    xt = nc.declare_dram_parameter("xt", [128, T // TC, N_HT, TC], BF16,
                                   isOutput=False)
    wqkvt = nc.declare_dram_parameter("wqkvt", [128, HL + 2, N_HT, 128], BF16,
                                      isOutput=False)
    wot = nc.declare_dram_parameter("wot", [128, HL, N_JT, 128], BF16,
                                    isOutput=False)
    ropes = nc.declare_dram_parameter("ropes", [128, T // TC, 4, TC], BF16,
                                      isOutput=False)
    # fp32 partial o-proj: [jt, j-in-tile, query-chunk, t]; host sums cores.
    out_t = nc.declare_dram_parameter("out_t", [N_JT, 128, N_CK, QC], F32,
                                      isOutput=True)

    with tile.TileContext(nc) as tc_ctx:
        _emit(tc_ctx, xt, wqkvt, wot, ropes, out_t)
    nc.finalize()
    return nc


def _host_inputs(hidden_states, w_qkv, w_o):
    """Shard + transpose inputs for the 8 cores; returns in_maps."""
    X = np.asarray(hidden_states, dtype=np.float32).reshape(T, HID)
    # [p, ch, ht, tc] tiled layout so every DMA line is contiguous.
    xt = np.ascontiguousarray(
        X.reshape(T // TC, TC, N_HT, 128).transpose(3, 0, 2, 1)
    ).astype(ml_dtypes.bfloat16)

    # RoPE tables in [d, t] layout with rotate-half sign folded into sin and
    # the attention scale folded into the Q tables.
    inv_freq = 1.0 / (ROPE_BASE ** (np.arange(0, D, 2, dtype=np.float32) / D))
    pos = np.arange(S, dtype=np.float32)
    freqs = np.outer(pos, inv_freq)                      # (S, D/2)
    emb = np.concatenate([freqs, freqs], axis=-1)        # (S, D)
    cos = np.cos(emb).T.astype(np.float32)               # (D, S)
    sin = np.sin(emb).T.astype(np.float32)
    sgn = np.concatenate([-np.ones(D // 2), np.ones(D // 2)]).astype(np.float32)
    sins = sgn[:, None] * sin
    cos_t = np.tile(cos, (1, B))                         # (D, T)
    sins_t = np.tile(sins, (1, B))
    scale = np.float32(D ** -0.5)
    ropes = np.stack([cos_t * scale, sins_t * scale, cos_t, sins_t], axis=0)
    ropes = np.ascontiguousarray(
        ropes.reshape(4, 128, T // TC, TC).transpose(1, 2, 0, 3)
    ).astype(ml_dtypes.bfloat16)

    w_qkv = np.asarray(w_qkv, dtype=np.float32)
    w_o = np.asarray(w_o, dtype=np.float32)
    q_sz = N_HEADS * D
    kv_sz = N_KV_HEADS * D
    in_maps = []
    for c in range(N_CORES):
        qr = w_qkv[c * HL * D:(c + 1) * HL * D]
        kr = w_qkv[q_sz + c * D:q_sz + (c + 1) * D]
        vr = w_qkv[q_sz + kv_sz + c * D:q_sz + kv_sz + (c + 1) * D]
        w_shard = np.concatenate([qr, kr, vr], axis=0)           # (768, HID)
        wqkvt_c = np.ascontiguousarray(
            w_shard.reshape(HL + 2, 128, N_HT, 128).transpose(3, 0, 2, 1)
        ).astype(ml_dtypes.bfloat16)
        # o-proj slice: this core's 512 attention rows, all 4096 columns,
        # laid out [d-part, hh, jt, j].
        wo_rows = w_o[:, c * HL * D:(c + 1) * HL * D]            # (4096, 512)
        wot_c = np.ascontiguousarray(
            wo_rows.T.reshape(HL, 128, N_JT, 128).transpose(1, 0, 2, 3)
        ).astype(ml_dtypes.bfloat16)
        in_maps.append({
            "xt": xt, "wqkvt": wqkvt_c, "wot": wot_c, "ropes": ropes,
        })
    return in_maps


def _run(hidden_states, w_qkv, w_o, trace=False, tmpdir=None):
    in_maps = _host_inputs(hidden_states, w_qkv, w_o)
    nc = _build_program()
    res = run_bass_kernel_spmd(nc, in_maps, list(range(N_CORES)),
                               trace=trace, tmpdir=tmpdir)
    acc = np.zeros((N_JT, 128, N_CK, QC), dtype=np.float32)
    for c in range(N_CORES):
        acc += np.asarray(res.results[c]["out_t"])
    # [jt, j, ck, t] -> [j(4096), t(4096)] -> [B, S, HID]
    out_jt = acc.reshape(N_JT * 128, N_CK * QC)
    out = np.ascontiguousarray(out_jt.T).reshape(B, S, HID).astype(np.float32)
    return out, res


def kernel(hidden_states, w_qkv, w_o):
    out, _ = _run(hidden_states, w_qkv, w_o, trace=False)
    return out
